# revision 27
# baseline (speedup 1.0000x reference)
"""InfoVAE loss kernel for Trainium2, data-parallel over batch on 8 NeuronCores.

Reference computation (see problem spec):
    recons_loss = mean((recons - x)^2)                    recons/x: [4096, 3, 64, 64]
    mmd  = km(pz,pz) + km(z,z) - 2*km(pz,z)               z/pz:     [4096, 128]
           where km(a,b) = mean_ij exp(-(|a_i-b_j|^2/D)/sigma), sigma = 2*D*z_var
    kld  = mean_n(-0.5 * sum_d(1 + lv - mu^2 - exp(lv)))
    loss = 5*recons_loss + 1.5*(1/N)*kld + 98.5/(N*(N-1))*mmd
    returns (loss, recons_loss, mmd, -kld)

MMD via factored Taylor moments instead of the N^2 pairwise kernel:
    k_ij = e^{-r_i} e^{-c_j} e^{p_ij},  r_i=|a_i|^2/2^16, c_j=|b_j|^2/2^16,
    p_ij = a_i.b_j/2^15.  |p| <~ 1e-3, so e^p = 1 + p + p^2/2 to ~1e-10:
      sum_ij k = S_A*S_B + (w_A.w_B)/2^15 + <G_A, G_B>/2^31
    with per-tensor weighted moments S = sum_i e^{-r_i}, w = sum_i e^{-r_i} a_i,
    G = sum_i e^{-r_i} a_i a_i^T.  All moments are additive over row blocks, so
    each core computes them for its own 512-row block; the host sums the 8
    block moments and assembles the three pair sums in float64.  Validated vs
    the f64 ground truth: mmd rel err ~1e-7 (the fp32 jax reference itself sits
    7.2e-3 from the f64 value; tolerance is 2e-2).

This removes every N^2 term: the kernel is a pure memory-bound stream of
recons/x (50 MB/core) with the tiny moment/KLD work hidden under the DMA.
"""

import numpy as np

N = 4096
D = 128
NCORES = 8
ROWS = N // NCORES            # 512 rows per core
IMG_F = 3 * 64 * 64           # 12288
P = 128
T_ROW = ROWS // P             # 4 row tiles per core
MSE_CHUNK = 2048
MSE_NCH = IMG_F // MSE_CHUNK  # 6
NMSE = T_ROW * MSE_NCH        # 24 chunks
NFULL = NMSE - 2              # last 2 chunks stream as 4-piece drains
NPIECE = 8
PIECE_W = MSE_CHUNK // 4      # 512
NMSECOL = NFULL + NPIECE      # accum columns
INV_2S = 1.0 / 2.0 ** 15
INV_S = 1.0 / 2.0 ** 16

# small_out column map
C_MSE = 0                     # mse partial sums
C_LV = NMSECOL                # sum(log_var)
C_MU2 = NMSECOL + 1           # sum(mu^2)
C_ELV = NMSECOL + 2           # sum(exp(log_var))
C_SZ = NMSECOL + 3            # S_z partial (per-partition)
C_SPZ = NMSECOL + 4           # S_pz partial
NSMALL = NMSECOL + 6
# gw_out column map: [G_z | w_z | G_pz | w_pz]
GW_W = D + 1                  # 129 columns per tensor
NGW = 2 * GW_W                # 258

_CACHE = {}


def _build():
    import concourse.bass as bass
    import concourse.tile as tile
    from concourse import bacc, mybir

    f32 = mybir.dt.float32
    AF = mybir.ActivationFunctionType
    ALU = mybir.AluOpType
    AX = mybir.AxisListType

    nc = bacc.Bacc("TRN2", target_bir_lowering=False, debug=False,
                   num_devices=NCORES)

    r_blk = nc.dram_tensor("r_blk", [ROWS, IMG_F], f32, kind="ExternalInput").ap()
    x_blk = nc.dram_tensor("x_blk", [ROWS, IMG_F], f32, kind="ExternalInput").ap()
    z_blk = nc.dram_tensor("z_blk", [ROWS, D], f32, kind="ExternalInput").ap()
    pz_blk = nc.dram_tensor("pz_blk", [ROWS, D], f32, kind="ExternalInput").ap()
    mu_blk = nc.dram_tensor("mu_blk", [ROWS, D], f32, kind="ExternalInput").ap()
    lv_blk = nc.dram_tensor("lv_blk", [ROWS, D], f32, kind="ExternalInput").ap()

    small_out = nc.dram_tensor("small_out", [P, NSMALL], f32,
                               kind="ExternalOutput").ap()
    gw_out = nc.dram_tensor("gw_out", [P, NGW], f32, kind="ExternalOutput").ap()

    with tile.TileContext(nc) as tc:
        with (
            tc.tile_pool(name="consts", bufs=1) as consts,
            tc.tile_pool(name="nat", bufs=1) as nat,
            tc.tile_pool(name="stream", bufs=6) as stream,
            tc.tile_pool(name="dpool", bufs=3) as dpool,
            tc.tile_pool(name="scratch", bufs=1) as scratch,
            tc.tile_pool(name="acc", bufs=1) as accp,
            tc.tile_pool(name="psmm", bufs=2, space="PSUM") as psmm,
        ):
            ones_col = consts.tile([P, 1], f32)
            nc.vector.memset(ones_col[:], 1.0)

            small_sb = accp.tile([P, NSMALL], f32)
            nc.vector.memset(small_sb[:, NSMALL - 1:NSMALL], 0.0)  # pad col
            gw_sb = accp.tile([P, NGW], f32)

            # small block loads on the idle SWDGE/Pool ring, row-contiguous
            # layout (row i = p*T_ROW + t -> 2KB contiguous per partition);
            # every consumer is a full-row reduction so the mapping is free.
            zb = nat.tile([P, T_ROW, D], f32)
            pzb = nat.tile([P, T_ROW, D], f32)
            mu_t = nat.tile([P, T_ROW, D], f32)
            lv_t = nat.tile([P, T_ROW, D], f32)
            nc.gpsimd.dma_start(out=zb[:], in_=z_blk.rearrange("(p t) d -> p t d", t=T_ROW))
            nc.gpsimd.dma_start(out=pzb[:], in_=pz_blk.rearrange("(p t) d -> p t d", t=T_ROW))
            nc.gpsimd.dma_start(out=mu_t[:], in_=mu_blk.rearrange("(p t) d -> p t d", t=T_ROW))
            nc.gpsimd.dma_start(out=lv_t[:], in_=lv_blk.rearrange("(p t) d -> p t d", t=T_ROW))

            rv = r_blk.rearrange("(t p) f -> p t f", p=P)
            xv = x_blk.rearrange("(t p) f -> p t f", p=P)

            # chunk schedule: 22 full 2048-wide chunks, then the last two
            # chunks as 512-wide pieces so the drain pipeline empties fast
            chunks = []
            for k in range(NFULL):
                t, c = divmod(k, MSE_NCH)
                chunks.append((k, t, c * MSE_CHUNK, MSE_CHUNK, ""))
            for j in range(NPIECE):
                k = NFULL + j // 4
                t, c = divmod(k, MSE_NCH)
                chunks.append((NFULL + j, t, c * MSE_CHUNK + (j % 4) * PIECE_W,
                               PIECE_W, "p"))

            inflight = {}

            def emit_load(i):
                col, t, lo, width, tagsuf = chunks[i]
                rt = stream.tile([P, width], f32, tag="rt" + tagsuf)
                xt = stream.tile([P, width], f32, tag="xt" + tagsuf)
                # split the two streams across the two HWDGE rings (SP + ACT)
                nc.sync.dma_start(out=rt[:], in_=rv[:, t, lo:lo + width])
                nc.scalar.dma_start(out=xt[:], in_=xv[:, t, lo:lo + width])
                inflight[i] = (rt, xt)

            def emit_compute(i):
                col, t, lo, width, tagsuf = chunks[i]
                rt, xt = inflight.pop(i)
                dt = dpool.tile([P, width], f32, tag="dt" + tagsuf)
                nc.vector.tensor_sub(dt[:], rt[:], xt[:])
                sc = scratch.tile([P, width], f32, tag="msq" + tagsuf)
                nc.scalar.activation(out=sc[:], in_=dt[:], func=AF.Square,
                                     accum_out=small_sb[:, C_MSE + col:C_MSE + col + 1])

            # ---- weighted-moment stages, spread thin across the stream ----
            r_z = consts.tile([P, T_ROW], f32, tag="rz")
            e_z = consts.tile([P, T_ROW], f32, tag="ez")
            r_pz = consts.tile([P, T_ROW], f32, tag="rpz")
            e_pz = consts.tile([P, T_ROW], f32, tag="epz")

            def emit_mom_rsq(nat_t, r_t, t):
                sq = scratch.tile([P, D], f32, tag="momsq")
                # Square(x/256) = x^2/65536 (scale is an exact pow2)
                nc.scalar.activation(out=sq[:], in_=nat_t[:, t, :],
                                     func=AF.Square, scale=1.0 / 256.0,
                                     accum_out=r_t[:, t:t + 1])

            def emit_mom_rest(nat_t, r_t, e_t, s_col, g_lo):
                """e = exp(-r), S partial -> s_col, G = sum_i e_i a_i a_i^T
                and w = sum_i e_i a_i -> gw_sb cols [g_lo : g_lo+129]."""
                nc.scalar.activation(out=e_t[:], in_=r_t[:], func=AF.Exp,
                                     scale=-1.0, accum_out=s_col)
                sc_t = nat.tile([P, T_ROW, D], f32, tag=f"sc{g_lo}")
                for t in range(T_ROW):
                    nc.vector.tensor_scalar_mul(sc_t[:, t, :], nat_t[:, t, :],
                                                e_t[:, t:t + 1])
                ps = psmm.tile([P, GW_W], f32, tag="mom")
                for t in range(T_ROW):
                    nc.tensor.matmul(ps[:, 0:D], lhsT=sc_t[:, t, :],
                                     rhs=nat_t[:, t, :],
                                     start=(t == 0), stop=(t == T_ROW - 1))
                for t in range(T_ROW):
                    nc.tensor.matmul(ps[:, D:D + 1], lhsT=sc_t[:, t, :],
                                     rhs=ones_col[:],
                                     start=(t == 0), stop=(t == T_ROW - 1))
                nc.vector.tensor_copy(gw_sb[:, g_lo:g_lo + GW_W], ps[:])

            def emit_kld_a():
                nc.vector.tensor_reduce(small_sb[:, C_LV:C_LV + 1], lv_t[:],
                                        axis=AX.XY, op=ALU.add)
                k1 = scratch.tile([P, T_ROW, D], f32, tag="ksc")
                nc.scalar.activation(out=k1[:], in_=mu_t[:], func=AF.Square,
                                     accum_out=small_sb[:, C_MU2:C_MU2 + 1])

            def emit_kld_b():
                k2 = scratch.tile([P, T_ROW, D], f32, tag="ksc")
                nc.scalar.activation(out=k2[:], in_=lv_t[:], func=AF.Exp,
                                     accum_out=small_sb[:, C_ELV:C_ELV + 1])

            side = {
                2: lambda: emit_mom_rsq(zb, r_z, 0),
                3: lambda: emit_mom_rsq(zb, r_z, 1),
                4: lambda: emit_mom_rsq(zb, r_z, 2),
                5: lambda: emit_mom_rsq(zb, r_z, 3),
                6: lambda: emit_mom_rest(zb, r_z, e_z,
                                         small_sb[:, C_SZ:C_SZ + 1], 0),
                7: lambda: emit_mom_rsq(pzb, r_pz, 0),
                8: lambda: emit_mom_rsq(pzb, r_pz, 1),
                9: lambda: emit_mom_rsq(pzb, r_pz, 2),
                10: lambda: emit_mom_rsq(pzb, r_pz, 3),
                11: lambda: emit_mom_rest(pzb, r_pz, e_pz,
                                          small_sb[:, C_SPZ:C_SPZ + 1], GW_W),
                12: emit_kld_a,
                13: emit_kld_b,
                14: lambda: nc.gpsimd.dma_start(out=gw_out, in_=gw_sb[:]),
            }

            # main loop: DMA issues run AHEAD chunks in front of compute so
            # the in-order ACT engine's issue stream never waits on a square
            NTOT = len(chunks)
            AHEAD = 4
            for i in range(min(AHEAD, NTOT)):
                emit_load(i)
            for i in range(NTOT):
                if i + AHEAD < NTOT:
                    emit_load(i + AHEAD)
                emit_compute(i)
                if i in side:
                    side[i]()

            # store issued from the ACT engine: no cross-engine hop after the
            # last accumulator read
            nc.scalar.dma_start(out=small_out, in_=small_sb[:])

    nc.compile()
    return nc


def get_nc():
    if "nc" not in _CACHE:
        _CACHE["nc"] = _build()
    return _CACHE["nc"]


def make_in_maps(recons, x, z, mu, log_var, prior_z):
    r2 = np.ascontiguousarray(recons, dtype=np.float32).reshape(N, IMG_F)
    x2 = np.ascontiguousarray(x, dtype=np.float32).reshape(N, IMG_F)
    z = np.ascontiguousarray(z, dtype=np.float32)
    pz = np.ascontiguousarray(prior_z, dtype=np.float32)
    mu = np.ascontiguousarray(mu, dtype=np.float32)
    lv = np.ascontiguousarray(log_var, dtype=np.float32)
    maps = []
    for c in range(NCORES):
        s = slice(c * ROWS, (c + 1) * ROWS)
        maps.append({
            "r_blk": r2[s], "x_blk": x2[s],
            "z_blk": z[s], "pz_blk": pz[s],
            "mu_blk": mu[s], "lv_blk": lv[s],
        })
    return maps


def combine(results):
    mse_sum = 0.0
    kld_total = 0.0
    S = {"z": 0.0, "pz": 0.0}
    w = {"z": np.zeros(D), "pz": np.zeros(D)}
    G = {"z": np.zeros((D, D)), "pz": np.zeros((D, D))}
    for res in results:
        sm = np.float64(res["small_out"])
        mse_sum += sm[:, C_MSE:C_MSE + NMSECOL].sum()
        kld_total += (ROWS * D + sm[:, C_LV].sum() - sm[:, C_MU2].sum()
                      - sm[:, C_ELV].sum())
        S["z"] += sm[:, C_SZ].sum()
        S["pz"] += sm[:, C_SPZ].sum()
        gw = np.float64(res["gw_out"])
        G["z"] += gw[:, 0:D]
        w["z"] += gw[:, D]
        G["pz"] += gw[:, GW_W:GW_W + D]
        w["pz"] += gw[:, GW_W + D]

    def pair_sum(a, b):
        return (S[a] * S[b] + (w[a] @ w[b]) * INV_2S
                + np.sum(G[a] * G[b]) * INV_2S * INV_2S * 0.5)

    s_pp = pair_sum("pz", "pz")
    s_zz = pair_sum("z", "z")
    s_pz = pair_sum("pz", "z")

    recons_loss = mse_sum / (N * IMG_F)
    mmd = (s_pp + s_zz - 2.0 * s_pz) / (float(N) * float(N))
    kld = -0.5 * kld_total / N
    beta, alpha, reg_w = 5.0, -0.5, 100.0
    loss = (beta * recons_loss
            + (1.0 - alpha) * (1.0 / N) * kld
            + (alpha + reg_w - 1.0) / (float(N) * (N - 1)) * mmd)
    return (np.float32(loss), np.float32(recons_loss),
            np.float32(mmd), np.float32(-kld))


def run(recons, x, z, mu, log_var, prior_z, trace=False):
    from concourse.bass_utils import run_bass_kernel_spmd
    nc = get_nc()
    in_maps = make_in_maps(recons, x, z, mu, log_var, prior_z)
    res = run_bass_kernel_spmd(nc, in_maps, list(range(NCORES)), trace=trace)
    return res


def kernel(recons, x, z, mu, log_var, prior_z):
    res = run(recons, x, z, mu, log_var, prior_z)
    return combine(res.results)


# revision 31
# speedup vs baseline: 1.0813x; 1.0813x over previous
"""InfoVAE loss kernel for Trainium2, data-parallel over batch on 8 NeuronCores.

Reference computation (see problem spec):
    recons_loss = mean((recons - x)^2)                    recons/x: [4096, 3, 64, 64]
    mmd  = km(pz,pz) + km(z,z) - 2*km(pz,z)               z/pz:     [4096, 128]
           where km(a,b) = mean_ij exp(-(|a_i-b_j|^2/D)/sigma), sigma = 2*D*z_var
    kld  = mean_n(-0.5 * sum_d(1 + lv - mu^2 - exp(lv)))
    loss = 5*recons_loss + 1.5*(1/N)*kld + 98.5/(N*(N-1))*mmd
    returns (loss, recons_loss, mmd, -kld)

MMD via factored Taylor moments instead of the N^2 pairwise kernel:
    k_ij = e^{-r_i} e^{-c_j} e^{p_ij},  r_i=|a_i|^2/2^16, c_j=|b_j|^2/2^16,
    p_ij = a_i.b_j/2^15.  |p| <~ 1e-3, so e^p = 1 + p + p^2/2 to ~1e-10:
      sum_ij k = S_A*S_B + (w_A.w_B)/2^15 + <G_A, G_B>/2^31
    with per-tensor weighted moments S = sum_i e^{-r_i}, w = sum_i e^{-r_i} a_i,
    G = sum_i e^{-r_i} a_i a_i^T.  All moments are additive over row blocks, so
    each core computes them for its own 512-row block; the host sums the 8
    block moments and assembles the three pair sums in float64.  Validated vs
    the f64 ground truth: mmd rel err ~1e-7 (the fp32 jax reference itself sits
    7.2e-3 from the f64 value; tolerance is 2e-2).

This removes every N^2 term: the kernel is a pure memory-bound stream of
recons/x (50 MB/core) with the tiny moment/KLD work hidden under the DMA.
"""

import numpy as np

N = 4096
D = 128
NCORES = 8
ROWS = N // NCORES            # 512 rows per core
IMG_F = 3 * 64 * 64           # 12288
P = 128
T_ROW = ROWS // P             # 4 row tiles per core
MSE_CHUNK = 2048
MSE_NCH = IMG_F // MSE_CHUNK  # 6
NMSE = T_ROW * MSE_NCH        # 24 chunks
NFULL = NMSE - 2              # last 2 chunks stream as 4-piece drains
NPIECE = 8
PIECE_W = MSE_CHUNK // 4      # 512
NMSECOL = NFULL + NPIECE      # accum columns
INV_2S = 1.0 / 2.0 ** 15
INV_S = 1.0 / 2.0 ** 16

# small_out column map
C_MSE = 0                     # mse partial sums
C_LV = NMSECOL                # sum(log_var)
C_MU2 = NMSECOL + 1           # sum(mu^2)
C_ELV = NMSECOL + 2           # sum(exp(log_var))
C_SZ = NMSECOL + 3            # S_z partial (per-partition)
C_SPZ = NMSECOL + 4           # S_pz partial
NSMALL = NMSECOL + 6
# gw_out column map: [G_z | w_z | G_pz | w_pz]
GW_W = D + 1                  # 129 columns per tensor
NGW = 2 * GW_W                # 258

_CACHE = {}


def _build():
    import concourse.bass as bass
    import concourse.tile as tile
    from concourse import bacc, mybir

    f32 = mybir.dt.float32
    AF = mybir.ActivationFunctionType
    ALU = mybir.AluOpType
    AX = mybir.AxisListType

    nc = bacc.Bacc("TRN2", target_bir_lowering=False, debug=False,
                   num_devices=NCORES)

    r_blk = nc.dram_tensor("r_blk", [ROWS, IMG_F], f32, kind="ExternalInput").ap()
    x_blk = nc.dram_tensor("x_blk", [ROWS, IMG_F], f32, kind="ExternalInput").ap()
    z_blk = nc.dram_tensor("z_blk", [ROWS, D], f32, kind="ExternalInput").ap()
    pz_blk = nc.dram_tensor("pz_blk", [ROWS, D], f32, kind="ExternalInput").ap()
    mu_blk = nc.dram_tensor("mu_blk", [ROWS, D], f32, kind="ExternalInput").ap()
    lv_blk = nc.dram_tensor("lv_blk", [ROWS, D], f32, kind="ExternalInput").ap()

    small_out = nc.dram_tensor("small_out", [P, NSMALL], f32,
                               kind="ExternalOutput").ap()
    gw_out = nc.dram_tensor("gw_out", [P, NGW], f32, kind="ExternalOutput").ap()

    with tile.TileContext(nc) as tc:
        with (
            tc.tile_pool(name="consts", bufs=1) as consts,
            tc.tile_pool(name="nat", bufs=1) as nat,
            tc.tile_pool(name="stream", bufs=7) as stream,
            tc.tile_pool(name="pstream", bufs=4) as pstream,
            tc.tile_pool(name="dpool", bufs=2) as dpool,
            tc.tile_pool(name="scratch", bufs=1) as scratch,
            tc.tile_pool(name="acc", bufs=1) as accp,
            tc.tile_pool(name="psmm", bufs=2, space="PSUM") as psmm,
        ):
            rv = r_blk.rearrange("(t p) f -> p t f", p=P)
            xv = x_blk.rearrange("(t p) f -> p t f", p=P)

            # chunk schedule: 22 full 2048-wide chunks, then the last two
            # chunks as 512-wide pieces so the drain pipeline empties fast
            chunks = []
            for k in range(NFULL):
                t, c = divmod(k, MSE_NCH)
                chunks.append((k, t, c * MSE_CHUNK, MSE_CHUNK, ""))
            for j in range(NPIECE):
                k = NFULL + j // 4
                t, c = divmod(k, MSE_NCH)
                chunks.append((NFULL + j, t, c * MSE_CHUNK + (j % 4) * PIECE_W,
                               PIECE_W, "p"))

            inflight = {}

            def emit_load(i):
                col, t, lo, width, tagsuf = chunks[i]
                pool = stream if not tagsuf else pstream
                rt = pool.tile([P, width], f32, tag="rt" + tagsuf)
                xt = pool.tile([P, width], f32, tag="xt" + tagsuf)
                # split the two streams across the two HWDGE rings (SP + ACT).
                # The first two xt's also go on the SP ring: the ACT ring's
                # first slot is the hoisted ACT_TABLE_LOAD (~1.3us), so its
                # first transfers start late.
                nc.sync.dma_start(out=rt[:], in_=rv[:, t, lo:lo + width])
                xeng = nc.sync if i < 2 else nc.scalar
                xeng.dma_start(out=xt[:], in_=xv[:, t, lo:lo + width])
                inflight[i] = (rt, xt)

            # first loads at the very top of program order: streaming starts
            # the moment the engines clear the entry barrier
            AHEAD = 6
            for i in range(AHEAD):
                emit_load(i)

            ones_col = consts.tile([P, 1], f32)
            nc.vector.memset(ones_col[:], 1.0)

            small_sb = accp.tile([P, NSMALL], f32)
            nc.vector.memset(small_sb[:, NSMALL - 1:NSMALL], 0.0)  # pad col
            gw_sb = accp.tile([P, NGW], f32)

            # small block loads on the idle SWDGE/Pool ring, row-contiguous
            # layout (row i = p*T_ROW + t -> 2KB contiguous per partition);
            # every consumer is a full-row reduction so the mapping is free.
            zb = nat.tile([P, T_ROW, D], f32)
            pzb = nat.tile([P, T_ROW, D], f32)
            mu_t = nat.tile([P, T_ROW, D], f32)
            lv_t = nat.tile([P, T_ROW, D], f32)
            nc.gpsimd.dma_start(out=zb[:], in_=z_blk.rearrange("(p t) d -> p t d", t=T_ROW))
            nc.gpsimd.dma_start(out=pzb[:], in_=pz_blk.rearrange("(p t) d -> p t d", t=T_ROW))
            nc.gpsimd.dma_start(out=mu_t[:], in_=mu_blk.rearrange("(p t) d -> p t d", t=T_ROW))
            nc.gpsimd.dma_start(out=lv_t[:], in_=lv_blk.rearrange("(p t) d -> p t d", t=T_ROW))

            def emit_compute(i):
                col, t, lo, width, tagsuf = chunks[i]
                rt, xt = inflight.pop(i)
                dt = dpool.tile([P, width], f32, tag="dt" + tagsuf)
                nc.vector.tensor_sub(dt[:], rt[:], xt[:])
                sc = scratch.tile([P, width], f32, tag="msq" + tagsuf)
                nc.scalar.activation(out=sc[:], in_=dt[:], func=AF.Square,
                                     accum_out=small_sb[:, C_MSE + col:C_MSE + col + 1])

            # ---- weighted-moment stages, spread thin across the stream ----
            r_z = consts.tile([P, T_ROW], f32, tag="rz")
            e_z = consts.tile([P, T_ROW], f32, tag="ez")
            r_pz = consts.tile([P, T_ROW], f32, tag="rpz")
            e_pz = consts.tile([P, T_ROW], f32, tag="epz")

            def emit_mom_rsq(nat_t, r_t, t):
                sq = scratch.tile([P, D], f32, tag="momsq")
                # Square(x/256) = x^2/65536 (scale is an exact pow2)
                nc.scalar.activation(out=sq[:], in_=nat_t[:, t, :],
                                     func=AF.Square, scale=1.0 / 256.0,
                                     accum_out=r_t[:, t:t + 1])

            def emit_mom_rest(nat_t, r_t, e_t, s_col, g_lo):
                """e = exp(-r), S partial -> s_col, G = sum_i e_i a_i a_i^T
                and w = sum_i e_i a_i -> gw_sb cols [g_lo : g_lo+129]."""
                nc.scalar.activation(out=e_t[:], in_=r_t[:], func=AF.Exp,
                                     scale=-1.0, accum_out=s_col)
                sc_t = nat.tile([P, T_ROW, D], f32, tag=f"sc{g_lo}")
                for t in range(T_ROW):
                    nc.vector.tensor_scalar_mul(sc_t[:, t, :], nat_t[:, t, :],
                                                e_t[:, t:t + 1])
                ps = psmm.tile([P, GW_W], f32, tag="mom")
                for t in range(T_ROW):
                    nc.tensor.matmul(ps[:, 0:D], lhsT=sc_t[:, t, :],
                                     rhs=nat_t[:, t, :],
                                     start=(t == 0), stop=(t == T_ROW - 1))
                for t in range(T_ROW):
                    nc.tensor.matmul(ps[:, D:D + 1], lhsT=sc_t[:, t, :],
                                     rhs=ones_col[:],
                                     start=(t == 0), stop=(t == T_ROW - 1))
                nc.vector.tensor_copy(gw_sb[:, g_lo:g_lo + GW_W], ps[:])

            def emit_kld_a():
                nc.vector.tensor_reduce(small_sb[:, C_LV:C_LV + 1], lv_t[:],
                                        axis=AX.XY, op=ALU.add)
                k1 = scratch.tile([P, T_ROW, D], f32, tag="ksc")
                nc.scalar.activation(out=k1[:], in_=mu_t[:], func=AF.Square,
                                     accum_out=small_sb[:, C_MU2:C_MU2 + 1])

            def emit_kld_b():
                k2 = scratch.tile([P, T_ROW, D], f32, tag="ksc")
                nc.scalar.activation(out=k2[:], in_=lv_t[:], func=AF.Exp,
                                     accum_out=small_sb[:, C_ELV:C_ELV + 1])

            side = {
                2: lambda: emit_mom_rsq(zb, r_z, 0),
                3: lambda: emit_mom_rsq(zb, r_z, 1),
                4: lambda: emit_mom_rsq(zb, r_z, 2),
                5: lambda: emit_mom_rsq(zb, r_z, 3),
                6: lambda: emit_mom_rest(zb, r_z, e_z,
                                         small_sb[:, C_SZ:C_SZ + 1], 0),
                7: lambda: emit_mom_rsq(pzb, r_pz, 0),
                8: lambda: emit_mom_rsq(pzb, r_pz, 1),
                9: lambda: emit_mom_rsq(pzb, r_pz, 2),
                10: lambda: emit_mom_rsq(pzb, r_pz, 3),
                11: lambda: emit_mom_rest(pzb, r_pz, e_pz,
                                          small_sb[:, C_SPZ:C_SPZ + 1], GW_W),
                12: emit_kld_a,
                13: emit_kld_b,
                14: lambda: nc.gpsimd.dma_start(out=gw_out, in_=gw_sb[:]),
            }

            # main loop: DMA issues run AHEAD chunks in front of compute so
            # the in-order ACT engine's issue stream never waits on a square
            NTOT = len(chunks)
            for i in range(NTOT):
                if i + AHEAD < NTOT:
                    emit_load(i + AHEAD)
                emit_compute(i)
                if i in side:
                    side[i]()

            # store issued from the ACT engine: no cross-engine hop after the
            # last accumulator read
            nc.scalar.dma_start(out=small_out, in_=small_sb[:])

    nc.compile()
    return nc


def get_nc():
    if "nc" not in _CACHE:
        _CACHE["nc"] = _build()
    return _CACHE["nc"]


def make_in_maps(recons, x, z, mu, log_var, prior_z):
    r2 = np.ascontiguousarray(recons, dtype=np.float32).reshape(N, IMG_F)
    x2 = np.ascontiguousarray(x, dtype=np.float32).reshape(N, IMG_F)
    z = np.ascontiguousarray(z, dtype=np.float32)
    pz = np.ascontiguousarray(prior_z, dtype=np.float32)
    mu = np.ascontiguousarray(mu, dtype=np.float32)
    lv = np.ascontiguousarray(log_var, dtype=np.float32)
    maps = []
    for c in range(NCORES):
        s = slice(c * ROWS, (c + 1) * ROWS)
        maps.append({
            "r_blk": r2[s], "x_blk": x2[s],
            "z_blk": z[s], "pz_blk": pz[s],
            "mu_blk": mu[s], "lv_blk": lv[s],
        })
    return maps


def combine(results):
    mse_sum = 0.0
    kld_total = 0.0
    S = {"z": 0.0, "pz": 0.0}
    w = {"z": np.zeros(D), "pz": np.zeros(D)}
    G = {"z": np.zeros((D, D)), "pz": np.zeros((D, D))}
    for res in results:
        sm = np.float64(res["small_out"])
        mse_sum += sm[:, C_MSE:C_MSE + NMSECOL].sum()
        kld_total += (ROWS * D + sm[:, C_LV].sum() - sm[:, C_MU2].sum()
                      - sm[:, C_ELV].sum())
        S["z"] += sm[:, C_SZ].sum()
        S["pz"] += sm[:, C_SPZ].sum()
        gw = np.float64(res["gw_out"])
        G["z"] += gw[:, 0:D]
        w["z"] += gw[:, D]
        G["pz"] += gw[:, GW_W:GW_W + D]
        w["pz"] += gw[:, GW_W + D]

    def pair_sum(a, b):
        return (S[a] * S[b] + (w[a] @ w[b]) * INV_2S
                + np.sum(G[a] * G[b]) * INV_2S * INV_2S * 0.5)

    s_pp = pair_sum("pz", "pz")
    s_zz = pair_sum("z", "z")
    s_pz = pair_sum("pz", "z")

    recons_loss = mse_sum / (N * IMG_F)
    mmd = (s_pp + s_zz - 2.0 * s_pz) / (float(N) * float(N))
    kld = -0.5 * kld_total / N
    beta, alpha, reg_w = 5.0, -0.5, 100.0
    loss = (beta * recons_loss
            + (1.0 - alpha) * (1.0 / N) * kld
            + (alpha + reg_w - 1.0) / (float(N) * (N - 1)) * mmd)
    return (np.float32(loss), np.float32(recons_loss),
            np.float32(mmd), np.float32(-kld))


def run(recons, x, z, mu, log_var, prior_z, trace=False):
    from concourse.bass_utils import run_bass_kernel_spmd
    nc = get_nc()
    in_maps = make_in_maps(recons, x, z, mu, log_var, prior_z)
    res = run_bass_kernel_spmd(nc, in_maps, list(range(NCORES)), trace=trace)
    return res


def kernel(recons, x, z, mu, log_var, prior_z):
    res = run(recons, x, z, mu, log_var, prior_z)
    return combine(res.results)
